# revision 12
# baseline (speedup 1.0000x reference)
"""Quincunx ConvBlock (GN->ReLU->qcConv x2 + skip 1x1 conv + GN, residual add)
on 8 TRN2 NeuronCores.

Sharding: batch (2) x H-quarters (4) -> 8 cores. Each core owns 64 output rows
of both cosets. GroupNorm stats need global (batch, group) sums -> two small
AllReduces over core groups {0..3} / {4..7} (GN1; skip-GN + GN2 merged).
The first AllReduce doubles as the core-skew barrier.

SBUF layout: 128 partitions = [coset0 ch 0..63 | coset1 ch 0..63].
Conv tiles store coset1 pre-shifted by (+1,+1) so each 2-row conv block is
4 full K=128 matmuls (one per 2x2 tap shift); the center taps fold into the
(0,0)/(1,1) shift weights. Plain-layout tiles feed stats and the skip conv.

PSUM drains alternate between the Scalar (ACT) and Vector (DVE) engines with
sum-accumulators harvesting GN stats; square-sums via DVE. The skip-path
residual is folded into the conv2 drain (S prescaled by a_s), so conv2 is
4 matmuls per block. ReLU applies interleave with conv matmul groups.
"""
import numpy as np
import ml_dtypes

import concourse.bass as bass
import concourse.tile as tile
from concourse import bacc, mybir
from concourse.bass_utils import run_bass_kernel_spmd

BF16 = ml_dtypes.bfloat16

B, C, H, W = 2, 64, 256, 256
G = 8
EPS = 1e-5
NCORES = 8
RPC = H // 4     # 64 owned rows per core
XR = RPC + 4     # conv x tile rows
HR = RPC + 2     # h tile rows (h rows R0-1 .. R1+1)
WP = W + 4       # padded row width for conv tiles

DT = mybir.dt.float32
BF = mybir.dt.bfloat16
AF = mybir.ActivationFunctionType
ALU = mybir.AluOpType

SHIFTS = ((0, 0), (0, 1), (1, 0), (1, 1))
# conv block-group sizes: tiny first groups for a fast post-AllReduce start
G1_GROUPS = [1, 2] + [3] * 9 + [2, 1]  # 33 blocks (66 h rows)
G2_GROUPS = [1, 2] + [3] * 9 + [1, 1]  # 32 blocks (64 out rows)
A1_CHUNKS = [(0, 4), (4, 12), (12, 20), (20, 28), (28, 36), (36, 44),
             (44, 52), (52, 60), (60, 68)]
A2_CHUNKS = [(0, 4), (4, 12), (12, 20), (20, 28), (28, 36), (36, 44),
             (44, 52), (52, 60), (60, 66)]

_CACHE = {}


def _rsqrt_newton(nc, out, v_ap, tmps):
    """out = rsqrt(v + EPS) via ACT sqrt + DVE reciprocal."""
    ve, sd = tmps["ve"], tmps["sd"]
    nc.vector.tensor_scalar(ve, v_ap, EPS, None, ALU.add)
    nc.scalar.activation(sd, ve, AF.Sqrt)
    nc.vector.reciprocal(out, sd)


def _gn_coeffs(nc, a, c, mean_ap, e2_ap, g_ap, b_ap, tmps):
    """a = g*rsqrt(var+eps), c = b - mean*a.  All APs may be [128, k]."""
    nc.vector.tensor_tensor(tmps["m"], mean_ap, mean_ap, ALU.mult)
    nc.vector.tensor_tensor(tmps["mm"], e2_ap, tmps["m"], ALU.subtract)
    _rsqrt_newton(nc, tmps["sd"], tmps["mm"], tmps)
    nc.vector.tensor_tensor(a, tmps["sd"], g_ap, ALU.mult)
    nc.vector.tensor_tensor(tmps["m"], mean_ap, a, ALU.mult)
    nc.vector.tensor_tensor(c, b_ap, tmps["m"], ALU.subtract)


def _build():
    nc = bacc.Bacc("TRN2", target_bir_lowering=False, debug=False,
                   num_devices=NCORES)

    xp_d = nc.dram_tensor("xp", [128, RPC, W], BF, kind="ExternalInput")
    x2_d = nc.dram_tensor("x2", [128, XR, WP], BF, kind="ExternalInput")
    cb_d = nc.dram_tensor("cbf", [128, 1408], BF, kind="ExternalInput")
    cf_d = nc.dram_tensor("cf32", [128, 18], DT, kind="ExternalInput")
    gt_d = nc.dram_tensor("gfull", [128, 128], DT, kind="ExternalInput")
    out_d = nc.dram_tensor("out", [128, RPC, W], BF, kind="ExternalOutput")

    with tile.TileContext(nc) as tc:
        with (
            tc.tile_pool(name="big", bufs=1) as big,
            tc.tile_pool(name="consts", bufs=1) as cpool,
            tc.tile_pool(name="stats", bufs=1) as spool,
            tc.tile_pool(name="trash", bufs=2) as tpool,
            tc.tile_pool(name="psum", bufs=2, space="PSUM") as pp,
            tc.tile_pool(name="psmall", bufs=1, space="PSUM") as ps,
            tc.tile_pool(name="dram", bufs=1, space="DRAM") as dp,
        ):
            X2 = big.tile([128, XR, WP], BF, tag="X2")
            XO = big.tile([128, RPC, W], BF, tag="XO")   # plain x, later OUT
            Hs = big.tile([128, HR, W], BF, tag="Hs")    # unshifted h
            H2 = big.tile([128, HR, WP], BF, tag="H2")   # conv-layout h
            S = big.tile([128, RPC, W], BF, tag="S")

            cb = cpool.tile([128, 1408], BF)
            cf = cpool.tile([128, 18], DT)
            gt = cpool.tile([128, 128], DT)

            w1 = cb[:, 0:512]
            w2 = cb[:, 512:1024]
            wsk = cb[:, 1024:1152]
            gind_bf = cb[:, 1280:1408]
            pp_g1, pp_b1, pp_bias1 = cf[:, 0:1], cf[:, 1:2], cf[:, 2:3]
            pp_g2, pp_b2, pp_bias2 = cf[:, 3:4], cf[:, 4:5], cf[:, 5:6]
            pp_gs, pp_bs, pp_biass = cf[:, 6:7], cf[:, 7:8], cf[:, 8:9]
            mxa, mxb = cf[:, 9:10], cf[:, 10:11]
            mha, mhb = cf[:, 11:12], cf[:, 12:13]
            # [g_s, g_2] / [b_s, b_2] pairs for the vectorized coeff chain
            pp_gsh, pp_bsh = cf[:, 13:15], cf[:, 15:17]

            RG = [[0, 1, 2, 3], [4, 5, 6, 7]]

            # dummy collective: absorbs CC cold-start + core-start skew so
            # the real AllReduces complete promptly
            dmy = spool.tile([8, 1], DT, tag="dmy", name="dmy")
            dmy_i = dp.tile([8, 1], DT, tag="dmyi", name="dmy_i")
            dmy_o = dp.tile([8, 1], DT, tag="dmyo", name="dmy_o")
            nc.vector.memset(dmy[:], 0.0)
            nc.sync.dma_start(dmy_i[:], dmy[:])
            nc.gpsimd.collective_compute("AllReduce", ALU.add,
                                         replica_groups=RG,
                                         ins=[dmy_i.opt()], outs=[dmy_o.opt()])

            # ---------------- DMAs in ----------------
            nc.sync.dma_start(cb[:], cb_d[:])
            nc.sync.dma_start(cf[:], cf_d[:])
            nc.sync.dma_start(gt[:], gt_d[:])
            for c in range(8):
                nc.sync.dma_start(XO[:, 8 * c:8 * (c + 1), :],
                                  xp_d[:, 8 * c:8 * (c + 1), :])
            for c in range(4):
                nc.sync.dma_start(X2[:, 17 * c:17 * (c + 1), :],
                                  x2_d[:, 17 * c:17 * (c + 1), :])

            def st(name, shape, dt=DT):
                return spool.tile(shape, dt, tag=name, name=name)

            NG1, NG2 = len(G1_GROUPS), len(G2_GROUPS)
            xsq = st("xsq", [128, 8])
            sdr = st("sdr", [128, 11])
            ssq = st("ssq", [128, 11])
            hdr = st("hdr", [128, NG1])
            hsq = st("hsq", [128, NG1])
            xm2 = st("xm2", [128, 1])
            stx = st("stx", [128, 2])
            st4 = st("st4", [128, 4])   # [sum_s, sum_h, sq_s, sq_h]
            s4r = st("s4r", [128, 4])
            g1b = st("g1b", [128, 2])
            g4b = st("g4b", [128, 4])
            a1, c1 = st("a1", [128, 1]), st("c1", [128, 1])
            ash = st("ash", [128, 2])   # [a_s, a_2]
            csh = st("csh", [128, 2])   # [c_s, c_2]
            dcon = st("dcon", [128, 1])
            tmps = {k: st("nt_" + k, [128, 2])
                    for k in ("ve", "sd", "y0", "t", "u", "w", "m", "mm")}
            t1 = {k: v[:, 0:1] for k, v in tmps.items()}
            t2 = {k: v[:, 0:2] for k, v in tmps.items()}

            ccx_i = dp.tile([128, 2], DT, tag="ccxi", name="ccx_i")
            ccx_o = dp.tile([4, 128, 2], DT, tag="ccxo", name="ccx_o")
            cc4_i = dp.tile([128, 4], DT, tag="cc4i", name="cc4_i")
            cc4_o = dp.tile([4, 128, 4], DT, tag="cc4o", name="cc4_o")

            # H2 pad columns (never written by drain copies) must be zero
            nc.gpsimd.memset(H2[0:64, :, 258:259], 0.0)
            nc.gpsimd.memset(H2[64:128, :, 2:3], 0.0)

            # ---------------- GN1 stats (ACT squares + PE sums) -------------
            for j in range(8):
                tr = tpool.tile([128, 8, 256], BF, tag="trash")
                nc.scalar.activation(tr[:], XO[:, 8 * j:8 * (j + 1), :],
                                     AF.Square, accum_out=xsq[:, j:j + 1])
            pm1 = ps.tile([128, 512], DT, tag="pm1")
            for t in range(32):
                nc.tensor.matmul(pm1[:, :], gind_bf[:],
                                 XO[:, 2 * t:2 * t + 2, :],
                                 start=(t == 0), stop=(t == 31),
                                 skip_group_check=True)

            # ---------------- AR1 (also the skew/cold-start barrier) --------
            nc.vector.reduce_sum(xm2[:], xsq[:], axis=mybir.AxisListType.X)
            pstat = ps.tile([128, 512], DT, tag="pstat", name="pstat")
            nc.tensor.matmul(pstat[:, 0:1], gt[:], xm2[:], start=True, stop=True)
            nc.vector.reduce_sum(stx[:, 0:1], pm1[:, :], axis=mybir.AxisListType.X)
            nc.vector.tensor_copy(stx[:, 1:2], pstat[:, 0:1])
            nc.sync.dma_start(ccx_i[:], stx[:])
            nc.gpsimd.collective_compute("AllGather", ALU.bypass,
                                         replica_groups=RG,
                                         ins=[ccx_i.opt()], outs=[ccx_o.opt()])
            gthx = st("gthx", [128, 8])
            for kk in range(4):
                nc.sync.dma_start(gthx[:, 2 * kk:2 * kk + 2], ccx_o[kk, :, :])
            nc.vector.tensor_tensor(tmps["t"], gthx[:, 0:2], gthx[:, 2:4],
                                    ALU.add)
            nc.vector.tensor_tensor(tmps["u"], gthx[:, 4:6], gthx[:, 6:8],
                                    ALU.add)
            nc.vector.tensor_tensor(stx[:], tmps["t"], tmps["u"], ALU.add)

            # ---------------- skip conv + S drains (ACT/DVE) + ssq ----------
            sk_groups = [3] * 10 + [2]
            bi = 0
            for g, nb in enumerate(sk_groups):
                pg = pp.tile([128, 1536], DT, tag="pg")
                for k in range(nb):
                    t = bi + k
                    nc.tensor.matmul(pg[:, 512 * k:512 * (k + 1)], wsk[:],
                                     XO[:, 2 * t:2 * t + 2, :],
                                     start=True, stop=True)
                if g % 2 == 0:
                    nc.scalar.activation(S[:, 2 * bi:2 * (bi + nb), :],
                                         pg[:, 0:512 * nb], AF.Identity,
                                         bias=pp_biass,
                                         accum_out=sdr[:, g:g + 1])
                else:
                    nc.vector.tensor_scalar(S[:, 2 * bi:2 * (bi + nb), :],
                                            pg[:, 0:512 * nb], pp_biass, 0.0,
                                            ALU.add, ALU.add,
                                            accum_out=sdr[:, g:g + 1])
                trs = tpool.tile([128, 6, 256], BF, tag="trs", name="trs")
                nc.vector.scalar_tensor_tensor(
                    trs[:, 0:nb * 2, :], S[:, 2 * bi:2 * (bi + nb), :], 1.0,
                    S[:, 2 * bi:2 * (bi + nb), :], ALU.mult, ALU.mult,
                    accum_out=ssq[:, g:g + 1])
                bi += nb

            # GN1 coeffs (DVE+ACT chain, gated on AR1 result)
            nc.vector.tensor_scalar(g1b[:], stx[:], 1.0 / (1 << 20),
                                    None, ALU.mult)
            _gn_coeffs(nc, a1[:], c1[:], g1b[:, 0:1], g1b[:, 1:2],
                       pp_g1, pp_b1, t1)

            # ---------------- apply1 on X2 (ACT) ----------------------------
            def emit_sliver1(r0, r1):
                nc.scalar.activation(X2[0:64, r0:r1, 2:3], X2[0:64, r0:r1, 2:3],
                                     AF.Relu, bias=c1[0:64], scale=a1[0:64])
                nc.scalar.activation(X2[64:128, r0:r1, 258:259],
                                     X2[64:128, r0:r1, 258:259],
                                     AF.Relu, bias=c1[64:128], scale=a1[64:128])

            emit_sliver1(0, 12)

            def emit_apply1(c):
                r0, r1 = A1_CHUNKS[c]
                nc.scalar.activation(X2[:, r0:r1, 3:258], X2[:, r0:r1, 3:258],
                                     AF.Relu, bias=c1[:], scale=a1[:])
                if c == 0:  # row 2 mask (x1 row -1 on core 0)
                    nc.vector.tensor_scalar(X2[:, 2:3, 2:259], X2[:, 2:3, 2:259],
                                            mxa, None, ALU.mult)
                if c == 8:  # row 66 mask (x0 row H on core 3)
                    nc.vector.tensor_scalar(X2[:, 66:67, 2:259], X2[:, 66:67, 2:259],
                                            mxb, None, ALU.mult)

            emit_apply1(0)
            emit_apply1(1)
            emit_sliver1(12, 68)
            emit_apply1(2)

            # ---------------- conv1 + drains + H2 DMAs + hsq ----------------
            def conv_block(pg, off, T, r0, wpk):
                for si, (dr, dc) in enumerate(SHIFTS):
                    nc.tensor.matmul(pg[:, off:off + 512],
                                     wpk[:, 128 * si:128 * (si + 1)],
                                     T[:, r0 + dr:r0 + dr + 2, 2 + dc:2 + dc + 256],
                                     start=(si == 0), stop=(si == 3))

            napply = 3
            bi = 0
            for g, nb in enumerate(G1_GROUPS):
                pg = pp.tile([128, 1536], DT, tag="pg")
                for k in range(nb):
                    t = bi + k
                    conv_block(pg, 512 * k, X2, 1 + 2 * t, w1)
                r0, r1 = 2 * bi, 2 * (bi + nb)   # Hs row range of this group
                # drain to Hs (alternating engines); accumulate sums over
                # owned h rows only (exclude storage rows 0 and 65)
                pieces = []
                if g == 0:
                    pieces = [(0, 1, False), (1, 2, True)]
                elif g == NG1 - 1:
                    pieces = [(r0, 65, True), (65, 66, False)]
                else:
                    pieces = [(r0, r1, True)]
                for (pa, pb, acc) in pieces:
                    src = pg[:, (pa - r0) * 256:(pb - r0) * 256]
                    ac = hdr[:, g:g + 1] if acc else None
                    if g % 2 == 0:
                        nc.scalar.activation(Hs[:, pa:pb, :], src, AF.Identity,
                                             bias=pp_bias1, accum_out=ac)
                    else:
                        nc.vector.tensor_scalar(Hs[:, pa:pb, :], src, pp_bias1,
                                                0.0, ALU.add, ALU.add,
                                                accum_out=ac)
                if napply < 9:
                    emit_apply1(napply)
                    napply += 1
                # conv-layout copies (DMA): h0 straight, h1 shifted (+1,+1)
                nc.sync.dma_start(H2[0:64, r0:r1, 2:258], Hs[0:64, r0:r1, :])
                h1b = min(r1, 65)
                nc.sync.dma_start(H2[64:128, r0 + 1:h1b + 1, 3:259],
                                  Hs[64:128, r0:h1b, :])
                # sum of h^2 over owned rows (DVE)
                ra, rb = max(r0, 1), min(r1, 65)
                trh = tpool.tile([128, 6, 256], BF, tag="trh", name="trh")
                nc.vector.scalar_tensor_tensor(
                    trh[:, 0:rb - ra, :], Hs[:, ra:rb, :], 1.0,
                    Hs[:, ra:rb, :], ALU.mult, ALU.mult,
                    accum_out=hsq[:, g:g + 1])
                bi += nb

            # ---------------- AR2 (skip-GN + GN2 merged) --------------------
            nc.vector.reduce_sum(st4[:, 0:1], sdr[:], axis=mybir.AxisListType.X)
            nc.vector.reduce_sum(st4[:, 1:2], hdr[:], axis=mybir.AxisListType.X)
            nc.vector.reduce_sum(st4[:, 2:3], ssq[:], axis=mybir.AxisListType.X)
            nc.vector.reduce_sum(st4[:, 3:4], hsq[:], axis=mybir.AxisListType.X)
            nc.tensor.matmul(pstat[:, 8:12], gt[:], st4[:], start=True, stop=True)
            nc.vector.tensor_copy(s4r[:], pstat[:, 8:12])
            nc.sync.dma_start(cc4_i[:], s4r[:])
            nc.gpsimd.collective_compute("AllGather", ALU.bypass,
                                         replica_groups=RG,
                                         ins=[cc4_i.opt()], outs=[cc4_o.opt()])
            gth4 = st("gth4", [128, 16])
            t4a = st("t4a", [128, 4])
            t4b = st("t4b", [128, 4])
            for kk in range(4):
                nc.sync.dma_start(gth4[:, 4 * kk:4 * kk + 4], cc4_o[kk, :, :])
            nc.vector.tensor_tensor(t4a[:], gth4[:, 0:4], gth4[:, 4:8], ALU.add)
            nc.vector.tensor_tensor(t4b[:], gth4[:, 8:12], gth4[:, 12:16],
                                    ALU.add)
            nc.vector.tensor_tensor(s4r[:], t4a[:], t4b[:], ALU.add)
            # vectorized coeff chain: cols [s, h]
            nc.vector.tensor_scalar(g4b[:], s4r[:], 1.0 / (1 << 20), None, ALU.mult)
            _gn_coeffs(nc, ash[:], csh[:], g4b[:, 0:2], g4b[:, 2:4],
                       pp_gsh, pp_bsh, t2)
            a2, c2 = ash[:, 1:2], csh[:, 1:2]
            nc.vector.tensor_tensor(dcon[:], pp_bias2, csh[:, 0:1], ALU.add)
            npre = 0

            def emit_prescale(upto_row):
                nonlocal npre
                while 16 * npre < upto_row:
                    nc.vector.tensor_scalar(S[:, 16 * npre:16 * (npre + 1), :],
                                            S[:, 16 * npre:16 * (npre + 1), :],
                                            ash[:, 0:1], None, ALU.mult)
                    npre += 1

            # ---------------- apply2 on H2 (ACT) ----------------------------
            def emit_sliver2(r0, r1):
                nc.scalar.activation(H2[0:64, r0:r1, 2:3], H2[0:64, r0:r1, 2:3],
                                     AF.Relu, bias=csh[0:64, 1:2],
                                     scale=ash[0:64, 1:2])
                nc.scalar.activation(H2[64:128, r0:r1, 258:259],
                                     H2[64:128, r0:r1, 258:259],
                                     AF.Relu, bias=csh[64:128, 1:2],
                                     scale=ash[64:128, 1:2])

            emit_sliver2(0, 12)

            def emit_apply2(c):
                r0, r1 = A2_CHUNKS[c]
                nc.scalar.activation(H2[:, r0:r1, 3:258], H2[:, r0:r1, 3:258],
                                     AF.Relu, bias=c2, scale=a2)
                if c == 0:  # row 1 mask (h1 row -1 on core 0)
                    nc.vector.tensor_scalar(H2[:, 1:2, 2:259], H2[:, 1:2, 2:259],
                                            mha, None, ALU.mult)
                if c == 8:  # row 65 mask (h0 row H on core 3)
                    nc.vector.tensor_scalar(H2[:, 65:66, 2:259], H2[:, 65:66, 2:259],
                                            mhb, None, ALU.mult)

            emit_apply2(0)
            emit_apply2(1)
            emit_sliver2(12, 66)
            emit_apply2(2)

            # ---------------- conv2 + fused residual drain + DMA out --------
            bi = 0
            napply = 3
            for g, nb in enumerate(G2_GROUPS):
                pg = pp.tile([128, 1536], DT, tag="pg")
                for k in range(nb):
                    t = bi + k
                    conv_block(pg, 512 * k, H2, 1 + 2 * t, w2)
                emit_prescale(2 * (bi + nb))
                # OUT = conv2psum + dcon + a_s*S   (DVE, one pass)
                nc.vector.scalar_tensor_tensor(
                    XO[:, 2 * bi:2 * (bi + nb), :], pg[:, 0:512 * nb], dcon[:],
                    S[:, 2 * bi:2 * (bi + nb), :], ALU.add, ALU.add)
                if napply < 9:
                    emit_apply2(napply)
                    napply += 1
                nc.sync.dma_start(out_d[:, 2 * bi:2 * (bi + nb), :],
                                  XO[:, 2 * bi:2 * (bi + nb), :])
                bi += nb

    nc.compile()
    return nc


def _get_nc():
    if "nc" not in _CACHE:
        _CACHE["nc"] = _build()
    return _CACHE["nc"]


# --------------------------------------------------------------------------
# host side
# --------------------------------------------------------------------------
def _pack_weights(w_center, w_corner):
    """[128, 512] packed lhsT weights: 4 shift blocks of [128,128]."""
    wp = np.zeros((128, 512), np.float32)
    wc = w_center.T
    wk = lambda a, b: w_corner[:, :, a, b].T
    for si, (a, b) in enumerate(SHIFTS):
        blk = wp[:, 128 * si:128 * (si + 1)]
        blk[0:64, 64:128] = wk(a, b)    # x0 -> out1, tap (a,b)
        blk[64:128, 0:64] = wk(a, b)    # x1(shifted) -> out0, tap (a,b)
        if (a, b) == (0, 0):
            blk[0:64, 0:64] = wc        # x0 -> out0 center
        if (a, b) == (1, 1):
            blk[64:128, 64:128] = wc    # x1(shifted) -> out1 center
    return wp


def kernel(x0, x1, g1, b1, w1_center, w1_corner, bias1,
           g2, b2, w2_center, w2_corner, bias2,
           w_skip, bias_skip, g_skip, beta_skip):
    x0 = np.asarray(x0, np.float32)
    x1 = np.asarray(x1, np.float32)

    # ---- constants ----
    cbf = np.zeros((128, 1408), np.float32)
    cbf[:, 0:512] = _pack_weights(np.asarray(w1_center), np.asarray(w1_corner))
    cbf[:, 512:1024] = _pack_weights(np.asarray(w2_center), np.asarray(w2_corner))
    wskf = np.zeros((128, 128), np.float32)
    wskf[0:64, 0:64] = np.asarray(w_skip).T
    wskf[64:128, 64:128] = np.asarray(w_skip).T
    cbf[:, 1024:1152] = wskf
    gind = np.zeros((128, 8), np.float32)
    for p in range(128):
        gind[p, (p % 64) // 8] = 1.0
    gfull = gind @ gind.T
    cbf[:, 1280:1408] = gfull
    cbf = cbf.astype(BF16)

    pp2 = lambda v: np.concatenate([np.asarray(v, np.float32)] * 2)
    base_cf = np.zeros((128, 18), np.float32)
    for i, v in enumerate((g1, b1, bias1, g2, b2, bias2,
                           g_skip, beta_skip, bias_skip)):
        base_cf[:, i] = pp2(v)
    base_cf[:, 9:13] = 1.0            # masks default
    base_cf[:, 13] = pp2(g_skip)      # [g_s, g_2] pair
    base_cf[:, 14] = pp2(g2)
    base_cf[:, 15] = pp2(beta_skip)   # [b_s, b_2] pair
    base_cf[:, 16] = pp2(b2)

    # ---- per-core inputs ----
    in_maps = []
    for core in range(NCORES):
        b, k = core // 4, core % 4
        r0 = RPC * k
        xp = np.zeros((128, RPC, W), np.float32)
        xp[0:64] = x0[b, :, r0:r0 + RPC, :]
        xp[64:128] = x1[b, :, r0:r0 + RPC, :]

        x2 = np.zeros((128, XR, WP), np.float32)
        lo, hi = r0 - 2, r0 + RPC + 2
        vlo, vhi = max(0, lo), min(H, hi)
        x2[0:64, vlo - lo:vhi - lo, 2:258] = x0[b, :, vlo:vhi, :]
        lo1, hi1 = r0 - 3, r0 + RPC + 1
        v1lo, v1hi = max(0, lo1), min(H, hi1)
        x2[64:128, v1lo - lo1:v1hi - lo1, 3:259] = x1[b, :, v1lo:v1hi, :]

        cf32 = base_cf.copy()
        if k == 0:
            cf32[64:128, 9] = 0.0    # mxa: X2 row 2 (x1 row -1)
            cf32[64:128, 11] = 0.0   # mha: H2 row 1 (h1 row -1)
        if k == 3:
            cf32[0:64, 10] = 0.0     # mxb: X2 row 66 (x0 row H)
            cf32[0:64, 12] = 0.0     # mhb: H2 row 65 (h0 row H)

        in_maps.append({
            "xp": xp.astype(BF16), "x2": x2.astype(BF16),
            "cbf": cbf, "cf32": cf32, "gfull": gfull,
        })

    nc = _get_nc()
    _CACHE["in_maps"] = in_maps
    res = run_bass_kernel_spmd(nc, in_maps, list(range(NCORES)))
    _CACHE["last_results"] = res

    out = np.empty((2, B, C, H, W), np.float32)
    for core in range(NCORES):
        b, k = core // 4, core % 4
        r0 = RPC * k
        arr = np.asarray(res.results[core]["out"]).astype(np.float32)
        out[0, b, :, r0:r0 + RPC, :] = arr[0:64]
        out[1, b, :, r0:r0 + RPC, :] = arr[64:128]
    return out
